# revision 7
# baseline (speedup 1.0000x reference)
"""Trainium2 Bass kernel for nn_BiasEncoder (Graphormer-style bias encoder).

Math (per edge e, with identity all-pairs scatter):
    out[e,k] = w_spatial[st[e],k] + (sum_d T[d, spt[e,d], k]) / max(st[e],1)
    T[d,v,k] = sum_h w_edge[v,h] * w_dis[d,h,k]

Device strategy (per core, data-parallel over 8 cores by edge blocks):
  - one-hot decode of spt/st values built as the *stationary* lhsT of PE
    matmuls: lhsT[(a,v), e] = (spt[e, 4q+a] == v), rhs = tiny T-table
    chunk [(a,v), k]; 6 chunk-matmuls accumulate psum[e, k].
  - one-hots are produced by DVE tensor_scalar is_equal (bf16) against a
    per-partition constant v = p%32, from a partition-replicated view of
    the transposed spt ("bspt") built by PE transpose + DRAM-mediated
    replicate DMA.
  - spatial table is pre-scaled by max(v,1) so a single psum * 1/max(st,1)
    rescale yields w_spatial[st] + contrib/dist exactly.
"""

import numpy as np
import ml_dtypes

B, N, H = 16, 128, 8
D = 20
NV = 32
E = B * N * N
NCORES = 8
EPC = E // NCORES          # edges per core

_PROG = {}


def _build_program(epc, js):
    import concourse.bacc as bacc
    import concourse.bass as bass
    import concourse.mybir as mybir
    import concourse.tile as tile
    from concourse._compat import axon_active

    dt = mybir.dt
    P = 128
    tcol = epc // P            # j columns per partition stripe
    assert tcol % js == 0 and js % 4 == 0
    nspan = tcol // js
    ng = js // 4               # transpose groups per span (4 j's each)
    FSs = 32 * js              # sptT row length in elems (= ng*128)
    SPAN_E = js * P            # edges per span

    nc = bacc.Bacc("TRN2", target_bir_lowering=False,
                   debug=not axon_active(), num_devices=NCORES)

    spt_d = nc.dram_tensor("spt", [epc, D], dt.int32, kind="ExternalInput")
    st_d = nc.dram_tensor("st", [epc], dt.int32, kind="ExternalInput")
    wedge_d = nc.dram_tensor("wedge", [NV, H], dt.float32, kind="ExternalInput")
    wdis_d = nc.dram_tensor("wdis", [D * H * H, 1], dt.float32, kind="ExternalInput")
    wsp_d = nc.dram_tensor("wsp", [21, H], dt.float32, kind="ExternalInput")
    identb_d = nc.dram_tensor("identb", [P, P], dt.bfloat16, kind="ExternalInput")
    identf_d = nc.dram_tensor("identf", [NV, NV], dt.float32, kind="ExternalInput")
    vvec_d = nc.dram_tensor("vvec", [P, 1], dt.float32, kind="ExternalInput")
    vmax_d = nc.dram_tensor("vmax", [P, 1], dt.float32, kind="ExternalInput")
    out_d = nc.dram_tensor("out", [epc, H], dt.float32, kind="ExternalOutput")
    # internal scratch
    sptT_d = nc.dram_tensor("sptTd", [nspan, 96, FSs], dt.bfloat16)
    t_d = nc.dram_tensor("tdram", [NV, D * H], dt.bfloat16)
    dbg = int(__import__("os").environ.get("K_DEBUG", "0"))
    if dbg:
        oh_d = nc.dram_tensor("ohdbg", [6, P, js * P], dt.bfloat16)
        dec_d = nc.dram_tensor("decdbg", [P, js * H], dt.float32)

    AP = bass.AP

    with tile.TileContext(nc) as tc:
        with tc.tile_pool(name="const", bufs=1) as cpool, \
             tc.tile_pool(name="ld", bufs=3) as lpool, \
             tc.tile_pool(name="big", bufs=2) as bpool, \
             tc.tile_pool(name="ps", bufs=2, space="PSUM") as ppool, \
             tc.tile_pool(name="pst", bufs=1, space="PSUM") as tbpool, \
             tc.tile_pool(name="psd", bufs=2, space="PSUM") as dpool:

            # ---- constants ----
            identb = cpool.tile([P, P], dt.bfloat16)
            nc.sync.dma_start(identb[:], identb_d[:])
            identf = cpool.tile([NV, NV], dt.float32)
            nc.sync.dma_start(identf[:], identf_d[:])
            vvec = cpool.tile([P, 1], dt.float32)
            nc.sync.dma_start(vvec[:], vvec_d[:])
            vmax = cpool.tile([P, 1], dt.float32)
            nc.sync.dma_start(vmax[:], vmax_d[:])

            # ---- T-table build ----
            we = cpool.tile([NV, H], dt.float32)
            nc.sync.dma_start(we[:], wedge_d[:])
            wet_ps = tbpool.tile([H, NV], dt.float32, tag="wet_ps")
            nc.tensor.transpose(wet_ps[:], we[:], identf[:])
            wet = cpool.tile([H, NV], dt.float32)
            nc.vector.tensor_copy(wet[:], wet_ps[:])
            wdis_sb = cpool.tile([H, D * H], dt.float32)
            nc.sync.dma_start(
                wdis_sb[:],
                AP(tensor=wdis_d[:].tensor, offset=0,
                   ap=[[H, H], [H * H, D], [1, H]]))
            tb_ps = tbpool.tile([NV, D * H], dt.float32, tag="tb_ps")
            nc.tensor.matmul(tb_ps[:], lhsT=wet[:], rhs=wdis_sb[:],
                             start=True, stop=True)
            tsb = cpool.tile([NV, D * H], dt.bfloat16)
            nc.vector.tensor_copy(tsb[:], tb_ps[:])
            nc.sync.dma_start(t_d[:], tsb[:])
            tq = []
            for q in range(5):
                t = cpool.tile([P, H], dt.bfloat16, tag=f"tq{q}")
                nc.sync.dma_start(
                    t[:],
                    AP(tensor=t_d[:].tensor, offset=4 * q * H,
                       ap=[[H, 4], [D * H, NV], [1, H]]))
                tq.append(t)
            # spatial table: t5[v,k] = w_spatial[v,k]*max(v,1), rows 21..127 = 0
            t5 = cpool.tile([P, H], dt.bfloat16, tag="tq5")
            nc.vector.memset(t5[:], 0)
            wspf = cpool.tile([21, H], dt.float32)
            nc.sync.dma_start(wspf[:], wsp_d[:])
            wspm = cpool.tile([21, H], dt.float32)
            nc.vector.tensor_scalar_mul(wspm[:], wspf[:], vmax[:][0:21, :])
            nc.vector.tensor_copy(t5[:][0:21, :], wspm[:])
            tq.append(t5)

            spt_v = spt_d[:].rearrange("(p t) d -> p t d", p=P)
            st_v = st_d[:].rearrange("(p t) -> p t", p=P)
            out_v = out_d[:].rearrange("(p t) k -> p t k", p=P)

            for s in range(nspan):
                # ---- loads ----
                spti = lpool.tile([P, js * D], dt.int32, tag="spti")
                nc.sync.dma_start(spti[:], spt_v[:, s * js:(s + 1) * js, :])
                sti = lpool.tile([P, js], dt.int32, tag="sti")
                nc.sync.dma_start(sti[:], st_v[:, s * js:(s + 1) * js])

                # ---- convert to bf16 staging [(g, slot, j')] ----
                s21 = lpool.tile([P, ng * 96], dt.bfloat16, tag="s21")
                nc.vector.tensor_copy(
                    AP(tensor=s21[:].tensor, offset=s21[:].offset,
                       ap=[[ng * 96, P], [96, ng], [1, 4], [4, D]]),
                    AP(tensor=spti[:].tensor, offset=spti[:].offset,
                       ap=[[js * D, P], [4 * D, ng], [D, 4], [1, D]]))
                nc.vector.tensor_copy(
                    AP(tensor=s21[:].tensor, offset=s21[:].offset + 80,
                       ap=[[ng * 96, P], [96, ng], [4, 4], [1, 4]]),
                    AP(tensor=sti[:].tensor, offset=sti[:].offset,
                       ap=[[js, P], [4, ng], [0, 4], [1, 4]]))

                # ---- transpose groups -> sptT in SBUF, then DRAM ----
                sptT = lpool.tile([96, FSs], dt.bfloat16, tag="sptT")
                for g in range(ng):
                    tp = ppool.tile([96, P], dt.bfloat16, tag="tp")
                    nc.tensor.transpose(
                        tp[:], s21[:][:, g * 96:(g + 1) * 96], identb[:])
                    nc.scalar.copy(sptT[:][:, g * P:(g + 1) * P], tp[:])
                nc.sync.dma_start(sptT_d[:][s], sptT[:])

                # ---- per-chunk: replicate -> compare -> 64 matmuls ----
                dec = dpool.tile([P, js * H], dt.float32, tag="dec")
                for q in range(6):
                    bs = bpool.tile([P, SPAN_E], dt.bfloat16, tag="bspt")
                    for jp in range(4):
                        src = AP(tensor=sptT_d[:].tensor,
                                 offset=(s * 96 + 16 * q + jp) * FSs,
                                 ap=[[4 * FSs, 4], [0, NV], [1, FSs]])
                        dst = AP(tensor=bs[:].tensor,
                                 offset=bs[:].offset + jp * FSs,
                                 ap=[[SPAN_E, P], [1, FSs]])
                        nc.sync.dma_start(dst, src)
                    oh = bpool.tile([P, SPAN_E], dt.bfloat16, tag="oh")
                    nc.vector.tensor_scalar(
                        out=oh[:], in0=bs[:], scalar1=vvec[:], scalar2=None,
                        op0=mybir.AluOpType.is_equal)
                    if dbg and s == 0:
                        nc.sync.dma_start(oh_d[:][q], oh[:])
                    for g in range(ng):
                        for jp in range(4):
                            jl = 4 * g + jp
                            nc.tensor.matmul(
                                dec[:][:, jl * H:(jl + 1) * H],
                                lhsT=oh[:][:, jp * FSs + g * P:
                                           jp * FSs + g * P + P],
                                rhs=tq[q][:],
                                start=(q == 0 and jl == 0),
                                stop=(q == 5 and g == ng - 1 and jp == 3),
                                skip_group_check=True)

                # ---- recip + combine + store ----
                rec = lpool.tile([P, js], dt.float32, tag="rec")
                nc.vector.tensor_copy(rec[:], sti[:])
                nc.vector.tensor_scalar_max(rec[:], rec[:], 1.0)
                nc.vector.reciprocal(rec[:], rec[:])
                rx8 = lpool.tile([P, js * H], dt.float32, tag="rx8")
                nc.vector.tensor_copy(
                    rx8[:],
                    AP(tensor=rec[:].tensor, offset=rec[:].offset,
                       ap=[[js, P], [1, js], [0, H]]))
                if dbg and s == 0:
                    dstg = lpool.tile([P, js * H], dt.float32, tag="dstg")
                    nc.vector.tensor_copy(dstg[:], dec[:])
                    nc.sync.dma_start(dec_d[:], dstg[:])
                stg = lpool.tile([P, js * H], dt.float32, tag="stg")
                nc.vector.tensor_tensor(
                    stg[:], dec[:], rx8[:], mybir.AluOpType.mult)
                nc.sync.dma_start(out_v[:, s * js:(s + 1) * js, :], stg[:])

    nc.compile()
    return nc


def _get_program(epc, js):
    key = (epc, js)
    if key not in _PROG:
        _PROG[key] = _build_program(epc, js)
    return _PROG[key]


def _consts():
    p = np.arange(128)
    identb = np.eye(128, dtype=np.float32).astype(ml_dtypes.bfloat16)
    identf = np.eye(NV, dtype=np.float32)
    vvec = (p % NV).astype(np.float32).reshape(128, 1)
    vmax = np.maximum(p % NV, 1).astype(np.float32).reshape(128, 1)
    return identb, identf, vvec, vmax


def _run_device(spt, st, w_edge, w_dis, w_spatial, epc=EPC, js=64):
    from concourse.bass_utils import run_bass_kernel_spmd
    nc = _get_program(epc, js)
    identb, identf, vvec, vmax = _consts()
    ncores = spt.shape[0] // epc
    in_maps = []
    for c in range(ncores):
        in_maps.append({
            "spt": np.ascontiguousarray(spt[c * epc:(c + 1) * epc]),
            "st": np.ascontiguousarray(st[c * epc:(c + 1) * epc]),
            "wedge": np.asarray(w_edge, np.float32),
            "wdis": np.asarray(w_dis, np.float32).reshape(D * H * H, 1),
            "wsp": np.asarray(w_spatial, np.float32),
            "identb": identb, "identf": identf,
            "vvec": vvec, "vmax": vmax,
        })
    res = run_bass_kernel_spmd(nc, in_maps, list(range(NCORES)))
    outs = [np.asarray(r["out"], np.float32) for r in res.results]
    return np.concatenate(outs, axis=0)


def _numpy_reference(spatial_types, shortest_path_types, graph_index, batch,
                     w_spatial, w_edge, w_edge_dis):
    """Faithful numpy port of the jax reference (scatter with drop semantics)."""
    src, dst = graph_index[0], graph_index[1]
    counts = np.bincount(batch, minlength=B)
    offsets = np.concatenate([[0], np.cumsum(counts)[:-1]]).astype(np.int64)
    g = batch[src]
    ls = src - offsets[g]
    ld = dst - offsets[g]
    valid = (ls >= 0) & (ls < N) & (ld >= 0) & (ld < N)
    gi, lsi, ldi = g[valid], ls[valid], ld[valid]

    bias = np.zeros((B, N, N, H), np.float32)
    np.add.at(bias, (gi, lsi, ldi), w_spatial[spatial_types[valid]])
    edge_enc = np.zeros((B, N, N, D, H), np.float32)
    np.add.at(edge_enc, (gi, lsi, ldi), w_edge[shortest_path_types[valid]])
    dist = np.zeros((B, N, N), np.float32)
    np.add.at(dist, (gi, lsi, ldi), spatial_types[valid].astype(np.float32))
    dist = np.clip(dist, 1.0, None)
    w_dis = w_edge_dis.reshape(D, H, H)
    edge_bias = np.einsum("bijdh,dhk->bijk", edge_enc, w_dis)
    return bias + edge_bias / dist[..., None]


def kernel(**inputs):
    spatial_types = np.asarray(inputs["spatial_types"])
    shortest_path_types = np.asarray(inputs["shortest_path_types"])
    graph_index = np.asarray(inputs["graph_index"])
    batch = np.asarray(inputs["batch"])
    w_spatial = np.asarray(inputs["w_spatial"], np.float32)
    w_edge = np.asarray(inputs["w_edge"], np.float32)
    w_edge_dis = np.asarray(inputs["w_edge_dis"], np.float32)

    # destination cell for each edge under general to_dense_adj semantics
    src, dst = graph_index[0].astype(np.int64), graph_index[1].astype(np.int64)
    counts = np.bincount(batch, minlength=B)
    offsets = np.concatenate([[0], np.cumsum(counts)[:-1]]).astype(np.int64)
    g = batch[src]
    ls = src - offsets[g]
    ld = dst - offsets[g]
    ok = (ls >= 0) & (ls < N) & (ld >= 0) & (ld < N)
    dest = g * N * N + ls * N + ld

    bijective = bool(ok.all()) and (np.bincount(dest, minlength=E).max() == 1)
    if not bijective:
        out = _numpy_reference(spatial_types, shortest_path_types, graph_index,
                               batch, w_spatial, w_edge, w_edge_dis)
        return out.astype(np.float32)

    if np.array_equal(dest, np.arange(E)):
        spt_in, st_in = shortest_path_types, spatial_types
    else:
        inv = np.empty(E, np.int64)
        inv[dest] = np.arange(E)
        spt_in = shortest_path_types[inv]
        st_in = spatial_types[inv]

    out = _run_device(np.asarray(spt_in, np.int32), np.asarray(st_in, np.int32),
                      w_edge, w_edge_dis, w_spatial)
    return out.reshape(B, N, N, H)
